# revision 14
# baseline (speedup 1.0000x reference)
"""Trainium2 Bass kernel for nn_Head (single-head causal attention, T=8).

Pure data parallel over 8 NeuronCores: per core x [4096, 8, 384] -> out
[4096, 8, 64]. The host marshals x into transposed bf16 layout
xT [128c, ns, 3, 512tok] so the device streams contiguous tiles and never
transposes activations on-chip.

Per 512-token supertile (tokens on partitions in 4 groups of 128), deeply
software-pipelined so every engine queue holds only work whose inputs
resolved >= 1 full iteration earlier (in-order sequencers otherwise wedge
the pipeline into a just-in-time fixed point):
  stage s    : DMA xT in (paired, 6KB/partition segments); fused
               Q^T|K^T = [Wq|Wk]^T @ x^T (3 MMs); V = x @ Wv (12 MMs);
               PSUM->SBUF CAST of qk (DVE, the ONLY recurring DVE op in
               front of the next CAST); K^T partition-shuffle via
               SBUF->SBUF DMA (gpsimd queue, nothing else on it)
  stage s-3  : scores: PSUM preloaded with +800*mask via a matmul
               (lhsT=mask^T, rhs=800*I), K^T@Q^T accumulates on top
               (uniform (128,128) PE tiles -- 64-row MMs cost a ~125ns
               reconfig); exp(x*SCALE - 800*SCALE) on ACT zeroes forbidden
               entries with no vector-engine mask op
  stage s-4  : out[q, 0:65] = S~ @ [V|1] (4 MMs); col 64 = softmax denom
  stage s-6  : PSUM->SBUF bf16 copy of [numerator|denominator]
  stage s-7  : DMA out, 2 supertiles per descriptor; host divides

bf16 matmul paths with f32 PSUM accumulation: rel error ~3e-3.
"""

import numpy as np
import ml_dtypes

import concourse.bass as bass
import concourse.mybir as mybir
from concourse import bacc
from concourse.tile import TileContext
from concourse.bass_utils import run_bass_kernel_spmd

N_CORES = 8
B_FULL = 32768
T = 8
C = 384
H = 64

BP = B_FULL // N_CORES       # batch rows per core
TOK = BP * T                 # tokens per core
ST = 512                     # tokens per supertile
G = ST // 128                # 128-token groups per supertile
NCH = C // 128               # contraction chunks
SCALE = float(C) ** -0.5
NV = 5                       # persistent [V|1] buffers (out stage at lag 4)
MB = 800.0                   # mask bias magnitude (exact in bf16)

BF16 = mybir.dt.bfloat16
F32 = mybir.dt.float32
AF = mybir.ActivationFunctionType

_nc_cache = {}


def _build_nc(ns: int):
    """Build the Bass module for `ns` supertiles per core."""
    assert ns % 2 == 0
    nc = bacc.Bacc("TRN2", target_bir_lowering=False, debug=False)

    xtd = nc.dram_tensor("xt", [128, ns, NCH, ST], BF16, kind="ExternalInput")
    wqkd = nc.dram_tensor("wqk", [128, NCH, 2 * H], BF16, kind="ExternalInput")
    wvd = nc.dram_tensor("wv", [128, NCH, H], BF16, kind="ExternalInput")
    masktt_d = nc.dram_tensor("masktt", [128, 128], BF16, kind="ExternalInput")
    eye_d = nc.dram_tensor("eyemb", [128, 128], BF16, kind="ExternalInput")
    od = nc.dram_tensor("out", [ns // 2, 128, 2, G, H + 1], BF16,
                        kind="ExternalOutput")

    with TileContext(nc) as tc:
        with (
            tc.tile_pool(name="const", bufs=1) as cpool,
            tc.tile_pool(name="xt", bufs=4) as xtpool,
            tc.tile_pool(name="qk", bufs=3) as qkpool,
            tc.tile_pool(name="sm", bufs=3) as smpool,
            tc.tile_pool(name="oo", bufs=3) as opool,
            tc.tile_pool(name="ps_qk", bufs=2, space="PSUM") as pqk,
            tc.tile_pool(name="ps_st", bufs=2, space="PSUM") as pst,
            tc.tile_pool(name="ps_v", bufs=2, space="PSUM") as pv,
            tc.tile_pool(name="ps_o", bufs=2, space="PSUM") as po,
        ):
            masktt = cpool.tile([128, 128], BF16)
            nc.sync.dma_start(masktt, masktt_d[:, :])
            eyemb = cpool.tile([128, 128], BF16)
            nc.sync.dma_start(eyemb, eye_d[:, :])
            wqk = cpool.tile([128, NCH, 2 * H], BF16)
            nc.sync.dma_start(wqk, wqkd[:, :, :])
            wv = cpool.tile([128, NCH, H], BF16)
            nc.sync.dma_start(wv, wvd[:, :, :])
            # persistent [V|1] tiles: ones column written once, V columns
            # rewritten every NV iterations (WAR tracked by the framework)
            v_tiles = [
                cpool.tile([128, G, H + 1], BF16, name=f"v{i}", tag=f"v{i}")
                for i in range(NV)
            ]
            for vt in v_tiles:
                nc.gpsimd.memset(vt[:, :, H:H + 1], 1.0)
            ebias = cpool.tile([128, 1], F32)
            nc.gpsimd.memset(ebias, -MB * SCALE)
            # persistent K^T pair tiles, full 128 partitions so the score MM
            # keeps the (128,128) PE tile config; rows 64:128 stay zero to
            # kill the K^T rows of the full-height rhs
            kt_tiles = [
                cpool.tile([128, 2, ST], BF16, name=f"kt{i}", tag=f"kt{i}")
                for i in range(3)
            ]
            for kt in kt_tiles:
                nc.gpsimd.memset(kt[H:128, :, :], 0.0)

            # per-stage state carried across pipeline iterations
            xt_pair = [None] * (ns // 2)
            qk_pair = [None] * (ns // 2)
            sm_sb = [None] * ns
            st_ps = [None] * ns
            v_ps = [None] * ns
            o_ps = [None] * ns
            o_pair = [None] * (ns // 2)

            for it in range(ns + 7):
                s = it          # projection stage
                s1 = it - 3     # scores stage
                s2 = it - 4     # output stage
                so = it - 6     # output PSUM->SBUF stage
                s3 = it - 7     # store stage

                if 0 <= s3 < ns and s3 % 2 == 1:
                    # store 2 supertiles bf16 [num|den] (1KB+/partition);
                    # deps resolved an iteration ago: never stalls sync
                    nc.sync.dma_start(od[s3 // 2], o_pair[s3 // 2])

                if s < ns and s % 2 == 0:
                    # load 2 supertiles of xT (partition-major dram: 6KB
                    # contiguous per partition); sync prefetches ahead
                    xt_pair[s // 2] = xtpool.tile([128, 2, NCH, ST], BF16,
                                                  name="xt", tag="xt")
                    nc.sync.dma_start(xt_pair[s // 2], xtd[:, s:s + 2])

                if s < ns:
                    xt_sb = xt_pair[s // 2][:, s % 2]
                    # fused Q^T|K^T: one MM per chunk, 128 PE columns
                    qk_ps = pqk.tile([128, ST], F32, name="qkps", tag="qkps")
                    for j in range(NCH):
                        nc.tensor.matmul(
                            qk_ps,
                            lhsT=wqk[:, j, :],
                            rhs=xt_sb[:, j, :],
                            start=(j == 0),
                            stop=(j == NCH - 1),
                        )

                # V(s) / scores(s1) / out(s2) interleaved on PE: the short
                # V MMs hide their weight loads under the longer streams
                for g in range(G):
                    if s < ns:
                        if g == 0:
                            v_ps[s] = pv.tile([128, G, H], F32,
                                              name="vps", tag="vps")
                        for j in range(NCH):
                            nc.tensor.matmul(
                                v_ps[s][:, g, :],
                                lhsT=xt_sb[:, j, g * 128:(g + 1) * 128],
                                rhs=wv[:, j, :],
                                start=(j == 0),
                                stop=(j == NCH - 1),
                            )
                    if 0 <= s1 < ns:
                        if g == 0:
                            st_ps[s1] = pst.tile([128, G, 128], F32,
                                                 name="stps", tag="stps")
                        # mask bias preload: st[s,q] = MB*mask[s,q]
                        nc.tensor.matmul(
                            st_ps[s1][:, g, :],
                            lhsT=masktt,
                            rhs=eyemb,
                            start=True,
                            stop=False,
                        )
                        nc.tensor.matmul(
                            st_ps[s1][:, g, :],
                            lhsT=kt_tiles[(s1 // 2) % 3][
                                :, s1 % 2, g * 128:(g + 1) * 128],
                            rhs=qk_pair[s1 // 2][
                                :, s1 % 2, g * 128:(g + 1) * 128],
                            start=False,
                            stop=True,
                        )
                    if 0 <= s2 < ns:
                        if g == 0:
                            o_ps[s2] = po.tile([128, G, H + 1], F32,
                                               name="ops", tag="ops")
                        nc.tensor.matmul(
                            o_ps[s2][:, g, :],
                            lhsT=sm_sb[s2][:, g, :],
                            rhs=v_tiles[s2 % NV][:, g, :],
                            start=True,
                            stop=True,
                        )

                if s < ns:
                    # PSUM->SBUF CAST (the only op in front of the next CAST
                    # in the DVE queue), then K^T shuffle down to base 0 on
                    # the otherwise-empty gpsimd queue
                    if s % 2 == 0:
                        qk_pair[s // 2] = qkpool.tile([128, 2, ST], BF16,
                                                      name="qk", tag="qk")
                    nc.vector.tensor_copy(qk_pair[s // 2][:, s % 2], qk_ps)
                    nc.gpsimd.dma_start(
                        kt_tiles[(s // 2) % 3][0:H, s % 2, :],
                        qk_pair[s // 2][H:2 * H, s % 2, :],
                    )

                if 0 <= s1 < ns:
                    # exp((scores + MB*mask)*SCALE - MB*SCALE): allowed
                    # entries exact, forbidden -> exp(-40.8) ~ 0
                    sm_sb[s1] = smpool.tile([128, G, 128], BF16,
                                            name="sm", tag="sm")
                    nc.scalar.activation(sm_sb[s1], st_ps[s1], AF.Exp,
                                         scale=SCALE, bias=ebias[:, 0:1])

                if s < ns:
                    # V PSUM->SBUF into the persistent [V|1] tile
                    nc.scalar.copy(v_tiles[s % NV][:, :, 0:H], v_ps[s])

                if 0 <= so < ns:
                    # [num|den] PSUM->SBUF bf16; normalization happens on
                    # the host. Lagged 2 iterations behind the out MMs so
                    # its wait never delays the next CAST in the DVE queue.
                    if so % 2 == 0:
                        o_pair[so // 2] = opool.tile([128, 2, G, H + 1],
                                                     BF16, name="o", tag="o")
                    nc.vector.tensor_copy(o_pair[so // 2][:, so % 2],
                                          o_ps[so])

    nc.finalize()
    return nc


def _consts():
    bf = ml_dtypes.bfloat16
    maskt = np.kron(
        np.eye(128 // T, dtype=np.float32),
        np.triu(np.ones((T, T), dtype=np.float32)),
    )
    masktt = np.ascontiguousarray(maskt.T).astype(bf)
    eyemb = (MB * np.eye(128, dtype=np.float32)).astype(bf)
    return masktt, eyemb


def _prepare(x, Wq, Wk, Wv):
    """Returns (nc, in_maps) for the full-size problem."""
    assert x.shape == (B_FULL, T, C), x.shape
    ns = TOK // ST
    if ns not in _nc_cache:
        _nc_cache[ns] = _build_nc(ns)
    nc = _nc_cache[ns]

    bf = ml_dtypes.bfloat16
    wqk_full = np.concatenate([Wq, Wk], axis=1)  # [C, 2H]
    wqk_h = np.ascontiguousarray(
        wqk_full.reshape(NCH, 128, 2 * H).transpose(1, 0, 2)
    ).astype(bf)
    wv_h = np.ascontiguousarray(
        Wv.reshape(NCH, 128, H).transpose(1, 0, 2)
    ).astype(bf)
    masktt, eyemb = _consts()

    # host-side marshalling: bf16 cast + transpose to [128c, ns, NCH, ST]
    xb = x.reshape(N_CORES, TOK // ST, ST, NCH, 128).astype(bf)
    in_maps = []
    for c in range(N_CORES):
        xs = np.ascontiguousarray(xb[c].transpose(3, 0, 2, 1))
        in_maps.append({
            "xt": xs, "wqk": wqk_h, "wv": wv_h,
            "masktt": masktt, "eyemb": eyemb,
        })
    return nc, in_maps


def _gather(results):
    ns = TOK // ST
    outs = []
    for r in results:
        arr = (
            np.asarray(r["out"])
            .reshape(ns // 2, 128, 2, G, H + 1)
            .transpose(0, 2, 3, 1, 4)
            .astype(np.float32)
        )
        out = arr[..., 0:H] / arr[..., H:H + 1]
        outs.append(out.reshape(BP, T, H))
    return np.concatenate(outs, axis=0)


def kernel(x, Wq, Wk, Wv):
    nc, in_maps = _prepare(x, Wq, Wk, Wv)
    res = run_bass_kernel_spmd(nc, in_maps, core_ids=list(range(N_CORES)))
    return _gather(res.results)
